# revision 21
# baseline (speedup 1.0000x reference)
"""Trainium2 Bass kernel for CustomGATConv (dense masked attention GNN layer).

  H = X @ W + b                       [8192, 64]
  S = H @ H.T ; S = where(A>0, S, -1e9)
  out = relu(softmax(S, -1) @ H)      [8192, 64]

Sharding: rows of the score matrix across 8 cores (1024 rows each);
H (N x 64, 0.6% of the FLOPs) is computed host-side during shard prep
and replicated to every core in both layouts the PE needs. All O(N^2)
work (scores, softmax, weighted sum - 99.4% of FLOPs) runs on-device.

Design (v7):
  - score matmuls in f32r (fp32 bits at ~1.5 cyc/col vs 4 for plain
    fp32): near-fp32 pre-exp precision at ~bf16 speed, K=64, N=512.
  - everything runs in "scores-transposed" space (score tile =
    [128 j-node partitions, 1024 core-row cols]) so the output matmul
    needs no on-chip transpose of the attention matrix.
  - the diagonal (scores reach ~192 and would overflow exp(s-64)) is
    killed PRE-exp by accumulating a static diag(-500) [128, 128] bf16
    tile onto the 128-col window of the 8 j-tiles that contain it.
  - exp on ScalarE in [128, 1024] chunks (PSUM 2-bank reads), bf16 out.
  - the A mask is applied POST-exp as a bf16 multiply on the DVE (2x
    packed mode). Off-diag scores obey |s| <= 99.6 so exp(s-64) never
    overflows; masked entries become exact zeros. The mask value is
    scm = e^{-max(|h_r|^2-C, 0)} (host-baked) instead of 1, so the
    [65, 1024] PSUM accumulator directly yields the scaled numerator
    and row-sum of the two-term diagonal softmax merge:
        out = (P*scm + h*scd) / (E*scm + scd)
    with scd host-provided; 1/den via ACT ln+exp (the DVE reciprocal
    is an 8x-slower iterative divide).
  - mask arrives as a host-interleaved bf16 tensor [128, 64*1024] in
    exactly the SBUF tile layout: contiguous streaming DMA, no
    DMA-transposes, no affine_selects on data. DMA issue order is
    hand-interleaved so every chunk lands before the loop needs it.

Per-core inputs are pre-rotated by the host (np.roll of columns by the
core's row offset) so the SPMD program is identical on every core.
"""

import sys
import numpy as np

for _p in ("/opt/trn_rl_repo",):
    if _p not in sys.path:
        sys.path.insert(0, _p)

import ml_dtypes

import concourse.bass as bass
import concourse.tile as tile
from concourse import bacc, mybir
from concourse.bass_utils import run_bass_kernel_spmd

N = 8192          # nodes
D = 200           # in dim
F = 64            # out dim
NCORES = 8
M = N // NCORES   # 1024 rows per core
P = 128           # partitions
C_SHIFT = 64.0    # global softmax shift for off-diagonal scores

f32 = mybir.dt.float32
f32r = mybir.dt.float32r
bf16 = mybir.dt.bfloat16
AF = mybir.ActivationFunctionType
ALU = mybir.AluOpType

MCH = 16          # mask chunks (4 j-tiles each)
JPC = 64 // MCH   # j-tiles per mask chunk


def build_kernel(nc, outT, hti, hsbi, mask, scd):
    """Emit the tile program. All arguments are DRAM APs."""
    from contextlib import ExitStack

    with ExitStack() as ctx:
        tc = nc._tc
        const = ctx.enter_context(tc.tile_pool(name="const", bufs=1))
        mkp = ctx.enter_context(tc.tile_pool(name="mk", bufs=8))
        work = ctx.enter_context(tc.tile_pool(name="work", bufs=4))
        ps_pool = ctx.enter_context(
            tc.tile_pool(name="ps", bufs=2, space="PSUM"))
        ps_out = ctx.enter_context(
            tc.tile_pool(name="ps_out", bufs=1, space="PSUM"))
        psm = ctx.enter_context(tc.tile_pool(name="psm", bufs=2, space="PSUM"))

        # ---- constants (all tiny) -----------------------------------------
        cbias = const.tile([P, 1], f32)           # -C bias for the exp
        nc.vector.memset(cbias[:], -C_SHIFT)
        dummy = const.tile([1, 1], f32)           # front-load the exp table
        nc.scalar.activation(dummy[:], cbias[0:1, 0:1], AF.Exp)

        identb = const.tile([P, P], bf16)         # bf16 identity
        nc.vector.memset(identb[:], 1.0)
        nc.gpsimd.affine_select(identb[:], identb[:], pattern=[[-1, P]],
                                base=0, channel_multiplier=1,
                                compare_op=ALU.is_equal, fill=0.0)
        dneg = const.tile([P, P], bf16)           # diag(-500)
        nc.vector.memset(dneg[:], 0.0)
        nc.gpsimd.affine_select(dneg[:], dneg[:], pattern=[[-1, P]],
                                base=0, channel_multiplier=1,
                                compare_op=ALU.not_equal, fill=-500.0)

        ones_rows = const.tile([1, F], f32)
        nc.vector.memset(ones_rows[:], 1.0)
        ones_row = const.tile([1, F], f32r)
        nc.vector.tensor_copy(ones_row[:], ones_rows[:])

        # ---- inputs: ht, hsb first, then the mask stream ------------------
        ht = const.tile([F, N], f32r)             # H.T (fp32 bits)
        hsb = const.tile([P, F * (F + 1)], bf16)  # per j-tile: [H_j | 1]
        scdt = const.tile([1, M], f32r)
        nc.gpsimd.dma_start(scdt[:], scd[:])

        mks = [None] * MCH

        def issue_mask(mc):
            # alternate between the sync HWDGE FIFO and the gpsimd SWDGE
            # FIFO: a chunk waiting for its buffer slot at one queue's head
            # must not block the next chunk's transfer
            mk = mkp.tile([P, JPC * M], bf16, tag="mk")
            eng = nc.sync if mc % 2 == 0 else nc.gpsimd
            eng.dma_start(mk[:], mask[:, mc * JPC * M : (mc + 1) * JPC * M])
            mks[mc] = mk

        # ht quarter q covers j-tiles 16q..16q+15; woven between the early
        # mask chunks so every transfer lands before the loop consumes it
        def ht_quarter(q):
            s = bass.ts(q, N // 4)
            nc.sync.dma_start(ht[:, s], hti[:, s])

        ht_quarter(0)
        nc.sync.dma_start(hsb[:], hsbi[:])
        issue_mask(0)
        issue_mask(1)
        issue_mask(2)
        issue_mask(3)
        ht_quarter(1)
        ht_quarter(2)
        ht_quarter(3)
        for mc in range(4, MCH):
            issue_mask(mc)

        # hts = ht[:, 0:M] * broadcast(scd): the diag-term numerator
        hts = const.tile([F, M], f32)
        for hs in (slice(0, 512), slice(512, M)):
            bb = psm.tile([F, 512], f32, tag="psm")
            nc.tensor.matmul(bb[:], ones_row[:], scdt[:, hs],
                             start=True, stop=True)
            nc.vector.tensor_copy(hts[:, hs], bb[:])
        nc.vector.tensor_mul(hts[:], ht[:, 0:M].bitcast(f32), hts[:])

        # ---- main attention loop ------------------------------------------
        po = ps_out.tile([F + 1, M], f32)

        for j in range(64):
            lhs = ht[:, j * P : (j + 1) * P]
            ps = ps_pool.tile([P, M], f32, tag="ps")
            dk = j < 8
            h0d = dk and j < 4          # diag window in half 0 / half 1
            nc.tensor.matmul(ps[:, 0:512], lhs, ht[:, 0:512],
                             start=True, stop=not h0d, skip_group_check=dk)
            nc.tensor.matmul(ps[:, 512:M], lhs, ht[:, 512:M],
                             start=True, stop=h0d, skip_group_check=dk)
            if dk:
                w = j * P
                nc.tensor.matmul(ps[:, w : w + P], identb[:], dneg[:],
                                 start=False, stop=True,
                                 skip_group_check=True)
            mk = mks[j // JPC]
            moff = (j % JPC) * M
            e = work.tile([P, M], bf16, tag="e")
            nc.scalar.activation(e[:], ps[:], AF.Exp, bias=cbias[:],
                                 scale=1.0)
            nc.vector.tensor_mul(e[:], e[:], mk[:, moff : moff + M])
            lh = hsb[:, j * (F + 1) : (j + 1) * (F + 1)]
            st, sp = (j == 0), (j == 63)
            nc.tensor.matmul(po[:, 0:512], lh, e[:, 0:512],
                             start=st, stop=sp, skip_group_check=True)
            nc.tensor.matmul(po[:, 512:M], lh, e[:, 512:M],
                             start=st, stop=sp, skip_group_check=True)

        # ---- tail: two-term merge, already mostly host-prepared ----------
        #   out = (po[0:64] + hts) / (po[64] + scd)
        fix = ctx.enter_context(tc.tile_pool(name="fix", bufs=1))

        den = fix.tile([1, M], f32)
        nc.vector.tensor_add(den[:], po[F : F + 1, :], scdt[:].bitcast(f32))
        nc.scalar.activation(den[:], den[:], AF.Ln)
        rcp = fix.tile([1, M], f32)
        nc.scalar.activation(rcp[:], den[:], AF.Exp, scale=-1.0)
        rcpr = fix.tile([1, M], f32r)
        nc.vector.tensor_copy(rcpr[:], rcp[:])

        res = fix.tile([F, M], f32, tag="mat")
        rb = fix.tile([F, M], f32, tag="mat2")
        osb = fix.tile([F, M], f32, tag="mat3")
        for ci, hs in enumerate((slice(0, 512), slice(512, M))):
            nc.vector.tensor_add(res[:, hs], po[0:F, hs], hts[:, hs])
            bb = psm.tile([F, 512], f32, tag="psm")
            nc.tensor.matmul(bb[:], ones_row[:], rcpr[:, hs],
                             start=True, stop=True)
            if ci == 0:
                nc.scalar.copy(rb[:, hs], bb[:])
            else:
                nc.vector.tensor_copy(rb[:, hs], bb[:])
            nc.vector.tensor_mul(res[:, hs], res[:, hs], rb[:, hs])
            nc.vector.tensor_scalar_max(osb[:, hs], res[:, hs], 0.0)
            nc.sync.dma_start(outT[:, hs], osb[:, hs])


_NC_CACHE = {}


def get_compiled():
    if "nc" not in _NC_CACHE:
        nc = bacc.Bacc("TRN2", target_bir_lowering=False, debug=False,
                       enable_asserts=True, num_devices=NCORES)
        hti = nc.dram_tensor("hti", [F, N], f32r, kind="ExternalInput").ap()
        hsbi = nc.dram_tensor("hsbi", [P, F * (F + 1)], bf16,
                              kind="ExternalInput").ap()
        mask = nc.dram_tensor("mask", [P, 64 * M], bf16,
                              kind="ExternalInput").ap()
        scd = nc.dram_tensor("scd", [1, M], f32r, kind="ExternalInput").ap()
        outT = nc.dram_tensor("outT", [F, M], f32, kind="ExternalOutput").ap()
        with tile.TileContext(nc) as tc:
            nc._tc = tc
            build_kernel(nc, outT, hti, hsbi, mask, scd)
        nc.compile()
        _NC_CACHE["nc"] = nc
    return _NC_CACHE["nc"]


def make_in_maps(X, A, W, b):
    X = np.ascontiguousarray(np.asarray(X, dtype=np.float32))
    A = np.asarray(A)
    if A.dtype != np.int32:
        A = A.astype(np.int32)
    W = np.asarray(W, dtype=np.float32)
    b = np.asarray(b, dtype=np.float32).reshape(1, F)

    # H and the per-row diagonal-merge scales (O(N*D*F) shard prep):
    #   d = |h_r|^2, t1 = d-C if A[r,r]>0 else -100, m = max(t1, 0)
    #   scm = e^{-m} (baked into the mask values), scd = e^{t1-m}
    H = (X @ W + b).astype(np.float32)
    dsq = np.einsum("ij,ij->i", H, H).astype(np.float32)
    adiag = np.diagonal(A).astype(np.float32)
    t1 = np.where(adiag > 0, dsq - np.float32(C_SHIFT), np.float32(-100.0))
    mvec = np.maximum(t1, 0.0).astype(np.float32)
    scm_all = np.exp(-mvec).astype(np.float32)
    scd_all = np.exp(t1 - mvec).astype(np.float32)

    # hsb: [H | 1] rows interleaved to the SBUF layout hsb[jj, j*65 + f]
    Hb = np.concatenate([H, np.ones((N, 1), np.float32)], axis=1)
    Hb = Hb.astype(ml_dtypes.bfloat16)            # [N, 65]

    rng = np.arange(M)
    in_maps = []
    for c in range(NCORES):
        r0 = c * M
        ht_c = np.ascontiguousarray(np.roll(H.T, -r0, axis=1))   # [64, N]
        hsb_c = np.ascontiguousarray(
            np.roll(Hb, -r0, axis=0).reshape(64, P, F + 1)
            .transpose(1, 0, 2)).reshape(P, 64 * (F + 1))
        blk = np.roll(A[r0 : r0 + M], -r0, axis=1)  # [M, N] int32, rotated
        blk[rng, rng] = 0                           # diag handled separately
        # bf16 mask scaled by scm, interleaved to the SBUF tile layout:
        # mk[jj, j*M + r] = scm[r] if edge(r, j*128+jj) else 0
        mu = ((blk != 0).astype(np.float32)
              * scm_all[r0 : r0 + M, None]).astype(ml_dtypes.bfloat16)
        mu = np.ascontiguousarray(
            mu.reshape(M, 64, P).transpose(2, 1, 0)).reshape(P, 64 * M)
        scd = scd_all[r0 : r0 + M].reshape(1, M)
        in_maps.append({"hti": ht_c, "hsbi": hsb_c, "mask": mu, "scd": scd})
    return in_maps


def kernel(X, A, W, b):
    nc = get_compiled()
    in_maps = make_in_maps(X, A, W, b)
    res = run_bass_kernel_spmd(nc, in_maps, list(range(NCORES)))
    outTs = [res.results[c]["outT"] for c in range(NCORES)]
    return np.ascontiguousarray(np.concatenate(outTs, axis=1).T)


# revision 22
# speedup vs baseline: 1.4286x; 1.4286x over previous
"""Trainium2 Bass kernel for CustomGATConv (dense masked attention GNN layer).

  H = X @ W + b                       [8192, 64]
  S = H @ H.T ; S = where(A>0, S, -1e9)
  out = relu(softmax(S, -1) @ H)      [8192, 64]

Sharding: rows of the score matrix across 8 cores (1024 rows each);
H (N x 64, 0.6% of the FLOPs) is computed host-side during shard prep
and replicated to every core in both layouts the PE needs. All O(N^2)
work (scores, softmax, weighted sum - 99.4% of FLOPs) runs on-device.

Design (v7):
  - score matmuls in f32r (fp32 bits at ~1.5 cyc/col vs 4 for plain
    fp32): near-fp32 pre-exp precision at ~bf16 speed, K=64, N=512.
  - everything runs in "scores-transposed" space (score tile =
    [128 j-node partitions, 1024 core-row cols]) so the output matmul
    needs no on-chip transpose of the attention matrix.
  - the diagonal (scores reach ~192 and would overflow exp(s-64)) is
    killed PRE-exp by accumulating a static diag(-500) [128, 128] bf16
    tile onto the 128-col window of the 8 j-tiles that contain it.
  - exp on ScalarE in [128, 1024] chunks (PSUM 2-bank reads), bf16 out.
  - the A mask is applied POST-exp as a bf16 multiply on the DVE (2x
    packed mode). Off-diag scores obey |s| <= 99.6 so exp(s-64) never
    overflows; masked entries become exact zeros. The mask value is
    scm = e^{-max(|h_r|^2-C, 0)} (host-baked) instead of 1, so the
    [65, 1024] PSUM accumulator directly yields the scaled numerator
    and row-sum of the two-term diagonal softmax merge:
        out = (P*scm + h*scd) / (E*scm + scd)
    with scd host-provided; 1/den via ACT ln+exp (the DVE reciprocal
    is an 8x-slower iterative divide).
  - mask arrives as a host-interleaved bf16 tensor [128, 64*1024] in
    exactly the SBUF tile layout: contiguous streaming DMA, no
    DMA-transposes, no affine_selects on data. DMA issue order is
    hand-interleaved so every chunk lands before the loop needs it.

Per-core inputs are pre-rotated by the host (np.roll of columns by the
core's row offset) so the SPMD program is identical on every core.
"""

import sys
import numpy as np

for _p in ("/opt/trn_rl_repo",):
    if _p not in sys.path:
        sys.path.insert(0, _p)

import ml_dtypes

import concourse.bass as bass
import concourse.tile as tile
from concourse import bacc, mybir
from concourse.bass_utils import run_bass_kernel_spmd

N = 8192          # nodes
D = 200           # in dim
F = 64            # out dim
NCORES = 8
M = N // NCORES   # 1024 rows per core
P = 128           # partitions
C_SHIFT = 64.0    # global softmax shift for off-diagonal scores

f32 = mybir.dt.float32
f32r = mybir.dt.float32r
bf16 = mybir.dt.bfloat16
AF = mybir.ActivationFunctionType
ALU = mybir.AluOpType

MCH = 16          # mask chunks (4 j-tiles each)
JPC = 64 // MCH   # j-tiles per mask chunk


def build_kernel(nc, outT, hti, hsbi, mask, scd):
    """Emit the tile program. All arguments are DRAM APs."""
    from contextlib import ExitStack

    with ExitStack() as ctx:
        tc = nc._tc
        const = ctx.enter_context(tc.tile_pool(name="const", bufs=1))
        mkp = ctx.enter_context(tc.tile_pool(name="mk", bufs=4))
        work = ctx.enter_context(tc.tile_pool(name="work", bufs=4))
        ps_pool = ctx.enter_context(
            tc.tile_pool(name="ps", bufs=2, space="PSUM"))
        ps_out = ctx.enter_context(
            tc.tile_pool(name="ps_out", bufs=1, space="PSUM"))
        psm = ctx.enter_context(tc.tile_pool(name="psm", bufs=2, space="PSUM"))

        # ---- constants (all tiny) -----------------------------------------
        cbias = const.tile([P, 1], f32)           # -C bias for the exp
        nc.vector.memset(cbias[:], -C_SHIFT)
        dummy = const.tile([1, 1], f32)           # front-load the exp table
        nc.scalar.activation(dummy[:], cbias[0:1, 0:1], AF.Exp)

        identb = const.tile([P, P], bf16)         # bf16 identity
        nc.vector.memset(identb[:], 1.0)
        nc.gpsimd.affine_select(identb[:], identb[:], pattern=[[-1, P]],
                                base=0, channel_multiplier=1,
                                compare_op=ALU.is_equal, fill=0.0)
        dneg = const.tile([P, P], bf16)           # diag(-500)
        nc.vector.memset(dneg[:], 0.0)
        nc.gpsimd.affine_select(dneg[:], dneg[:], pattern=[[-1, P]],
                                base=0, channel_multiplier=1,
                                compare_op=ALU.not_equal, fill=-500.0)

        ones_rows = const.tile([1, F], f32)
        nc.vector.memset(ones_rows[:], 1.0)
        ones_row = const.tile([1, F], f32r)
        nc.vector.tensor_copy(ones_row[:], ones_rows[:])

        # ---- inputs: ht, hsb first, then the mask stream ------------------
        ht = const.tile([F, N], f32r)             # H.T (fp32 bits)
        hsb = const.tile([P, F * (F + 1)], bf16)  # per j-tile: [H_j | 1]
        scdt = const.tile([1, M], f32r)
        nc.gpsimd.dma_start(scdt[:], scd[:])

        mks = [None] * MCH

        def issue_mask(mc):
            mk = mkp.tile([P, JPC * M], bf16, tag="mk")
            nc.sync.dma_start(
                mk[:], mask[:, mc * JPC * M : (mc + 1) * JPC * M])
            mks[mc] = mk

        # ht quarter q covers j-tiles 16q..16q+15; woven between the early
        # mask chunks so every transfer lands before the loop consumes it
        def ht_quarter(q):
            s = bass.ts(q, N // 4)
            nc.sync.dma_start(ht[:, s], hti[:, s])

        ht_quarter(0)
        ht_quarter(1)
        nc.sync.dma_start(hsb[:], hsbi[:])
        issue_mask(0)
        ht_quarter(2)
        ht_quarter(3)
        for mc in range(1, MCH):
            issue_mask(mc)

        # hts = ht[:, 0:M] * broadcast(scd): the diag-term numerator
        hts = const.tile([F, M], f32)
        for hs in (slice(0, 512), slice(512, M)):
            bb = psm.tile([F, 512], f32, tag="psm")
            nc.tensor.matmul(bb[:], ones_row[:], scdt[:, hs],
                             start=True, stop=True)
            nc.vector.tensor_copy(hts[:, hs], bb[:])
        nc.vector.tensor_mul(hts[:], ht[:, 0:M].bitcast(f32), hts[:])

        # ---- main attention loop ------------------------------------------
        po = ps_out.tile([F + 1, M], f32)

        for j in range(64):
            lhs = ht[:, j * P : (j + 1) * P]
            ps = ps_pool.tile([P, M], f32, tag="ps")
            dk = j < 8
            h0d = dk and j < 4          # diag window in half 0 / half 1
            nc.tensor.matmul(ps[:, 0:512], lhs, ht[:, 0:512],
                             start=True, stop=not h0d, skip_group_check=dk)
            nc.tensor.matmul(ps[:, 512:M], lhs, ht[:, 512:M],
                             start=True, stop=h0d, skip_group_check=dk)
            if dk:
                w = j * P
                nc.tensor.matmul(ps[:, w : w + P], identb[:], dneg[:],
                                 start=False, stop=True,
                                 skip_group_check=True)
            mk = mks[j // JPC]
            moff = (j % JPC) * M
            e = work.tile([P, M], bf16, tag="e")
            nc.scalar.activation(e[:], ps[:], AF.Exp, bias=cbias[:],
                                 scale=1.0)
            nc.vector.tensor_mul(e[:], e[:], mk[:, moff : moff + M])
            lh = hsb[:, j * (F + 1) : (j + 1) * (F + 1)]
            st, sp = (j == 0), (j == 63)
            nc.tensor.matmul(po[:, 0:512], lh, e[:, 0:512],
                             start=st, stop=sp, skip_group_check=True)
            nc.tensor.matmul(po[:, 512:M], lh, e[:, 512:M],
                             start=st, stop=sp, skip_group_check=True)

        # ---- tail: two-term merge, already mostly host-prepared ----------
        #   out = (po[0:64] + hts) / (po[64] + scd)
        fix = ctx.enter_context(tc.tile_pool(name="fix", bufs=1))

        den = fix.tile([1, M], f32)
        nc.vector.tensor_add(den[:], po[F : F + 1, :], scdt[:].bitcast(f32))
        nc.scalar.activation(den[:], den[:], AF.Ln)
        rcp = fix.tile([1, M], f32)
        nc.scalar.activation(rcp[:], den[:], AF.Exp, scale=-1.0)
        rcpr = fix.tile([1, M], f32r)
        nc.vector.tensor_copy(rcpr[:], rcp[:])

        res = fix.tile([F, M], f32, tag="mat")
        rb = fix.tile([F, M], f32, tag="mat2")
        osb = fix.tile([F, M], f32, tag="mat3")
        for ci, hs in enumerate((slice(0, 512), slice(512, M))):
            nc.vector.tensor_add(res[:, hs], po[0:F, hs], hts[:, hs])
            bb = psm.tile([F, 512], f32, tag="psm")
            nc.tensor.matmul(bb[:], ones_row[:], rcpr[:, hs],
                             start=True, stop=True)
            if ci == 0:
                nc.scalar.copy(rb[:, hs], bb[:])
            else:
                nc.vector.tensor_copy(rb[:, hs], bb[:])
            nc.vector.tensor_mul(res[:, hs], res[:, hs], rb[:, hs])
            nc.vector.tensor_scalar_max(osb[:, hs], res[:, hs], 0.0)
            nc.sync.dma_start(outT[:, hs], osb[:, hs])


_NC_CACHE = {}


def get_compiled():
    if "nc" not in _NC_CACHE:
        nc = bacc.Bacc("TRN2", target_bir_lowering=False, debug=False,
                       enable_asserts=True, num_devices=NCORES)
        hti = nc.dram_tensor("hti", [F, N], f32r, kind="ExternalInput").ap()
        hsbi = nc.dram_tensor("hsbi", [P, F * (F + 1)], bf16,
                              kind="ExternalInput").ap()
        mask = nc.dram_tensor("mask", [P, 64 * M], bf16,
                              kind="ExternalInput").ap()
        scd = nc.dram_tensor("scd", [1, M], f32r, kind="ExternalInput").ap()
        outT = nc.dram_tensor("outT", [F, M], f32, kind="ExternalOutput").ap()
        with tile.TileContext(nc) as tc:
            nc._tc = tc
            build_kernel(nc, outT, hti, hsbi, mask, scd)
        nc.compile()
        _NC_CACHE["nc"] = nc
    return _NC_CACHE["nc"]


def make_in_maps(X, A, W, b):
    X = np.ascontiguousarray(np.asarray(X, dtype=np.float32))
    A = np.asarray(A)
    if A.dtype != np.int32:
        A = A.astype(np.int32)
    W = np.asarray(W, dtype=np.float32)
    b = np.asarray(b, dtype=np.float32).reshape(1, F)

    # H and the per-row diagonal-merge scales (O(N*D*F) shard prep):
    #   d = |h_r|^2, t1 = d-C if A[r,r]>0 else -100, m = max(t1, 0)
    #   scm = e^{-m} (baked into the mask values), scd = e^{t1-m}
    H = (X @ W + b).astype(np.float32)
    dsq = np.einsum("ij,ij->i", H, H).astype(np.float32)
    adiag = np.diagonal(A).astype(np.float32)
    t1 = np.where(adiag > 0, dsq - np.float32(C_SHIFT), np.float32(-100.0))
    mvec = np.maximum(t1, 0.0).astype(np.float32)
    scm_all = np.exp(-mvec).astype(np.float32)
    scd_all = np.exp(t1 - mvec).astype(np.float32)

    # hsb: [H | 1] rows interleaved to the SBUF layout hsb[jj, j*65 + f]
    Hb = np.concatenate([H, np.ones((N, 1), np.float32)], axis=1)
    Hb = Hb.astype(ml_dtypes.bfloat16)            # [N, 65]

    rng = np.arange(M)
    in_maps = []
    for c in range(NCORES):
        r0 = c * M
        ht_c = np.ascontiguousarray(np.roll(H.T, -r0, axis=1))   # [64, N]
        hsb_c = np.ascontiguousarray(
            np.roll(Hb, -r0, axis=0).reshape(64, P, F + 1)
            .transpose(1, 0, 2)).reshape(P, 64 * (F + 1))
        blk = np.roll(A[r0 : r0 + M], -r0, axis=1)  # [M, N] int32, rotated
        blk[rng, rng] = 0                           # diag handled separately
        # bf16 mask scaled by scm, interleaved to the SBUF tile layout:
        # mk[jj, j*M + r] = scm[r] if edge(r, j*128+jj) else 0
        mu = ((blk != 0).astype(np.float32)
              * scm_all[r0 : r0 + M, None]).astype(ml_dtypes.bfloat16)
        mu = np.ascontiguousarray(
            mu.reshape(M, 64, P).transpose(2, 1, 0)).reshape(P, 64 * M)
        scd = scd_all[r0 : r0 + M].reshape(1, M)
        in_maps.append({"hti": ht_c, "hsbi": hsb_c, "mask": mu, "scd": scd})
    return in_maps


def kernel(X, A, W, b):
    nc = get_compiled()
    in_maps = make_in_maps(X, A, W, b)
    res = run_bass_kernel_spmd(nc, in_maps, list(range(NCORES)))
    outTs = [res.results[c]["outT"] for c in range(NCORES)]
    return np.ascontiguousarray(np.concatenate(outTs, axis=1).T)
